# revision 8
# baseline (speedup 1.0000x reference)
"""Trainium2 Bass kernel for nn_CPAMDec_Mix (dual cross-attention, CPAM decoder).

Math (per batch element n):
    q_i = (wq_i @ x_i + bq_i)            # (D, HW)   1x1 conv query
    k_i = y_i @ wk_i.T + bk_i            # (K, D)    linear key
    v_i = y_i @ wv_i.T + bv_i            # (K, C)    linear value
    e   = | q_1.T k_1.T - q_2.T k_2.T |  # (HW, K)
    a   = softmax_K(e)
    out_i = scale * (v_i.T @ a.T) + x_i  # (C, HW)

Sharding: pure data parallel, one batch element per NeuronCore (N=8, 8 cores).
All weights replicated.  Host-side marshaling pre-transposes the small weight
matrices / y tensors so the contraction dim (C) lands on SBUF partitions.

On-chip layout per core (everything streamed over pixel tiles of L=512):
    E^T (K x L) layout keeps softmax results directly usable as the moving
    operand of the output matmul (contract over K).  Softmax over K (the
    partition dim) is done with ones-matmuls: S = 1.T @ exp(E), then
    R = 1/S broadcast back over K partitions with another ones-matmul.
    exp() needs no max-subtraction: energies are |.| >= 0 and bounded
    (~20 for this operator scale), far from fp32 overflow.
    Matmuls run as float32r (fp32 bits, replicated fast path: 1 PE
    cycle/row for moving >= 256 instead of 4 for plain fp32).  The BIR
    verifier requires every f32r matmul operand to be produced as f32r,
    so matmul-feeding DRAM tensors/tiles are declared f32r end-to-end;
    the residual add reads the x tiles bitcast back to f32 (exact bits).
"""

import numpy as np

N, C, H, W, K = 8, 512, 64, 64, 64
HW = H * W          # 4096
D = C // 4          # 128
L = 512             # pixel tile size
NT = HW // L        # 8 tiles
NCH = C // 128      # 4 contraction chunks
P = 128

_CACHE = {}


def _build():
    from contextlib import ExitStack

    import concourse.tile as tile
    from concourse import bacc, mybir

    f32 = mybir.dt.float32
    f32r = mybir.dt.float32r
    AF = mybir.ActivationFunctionType
    ALU = mybir.AluOpType

    nc = bacc.Bacc("TRN2", target_bir_lowering=False, debug=False)

    def din(name, shape, dt=f32):
        return nc.dram_tensor(name, shape, dt, kind="ExternalInput").ap()

    def dout(name, shape):
        return nc.dram_tensor(name, shape, f32, kind="ExternalOutput").ap()

    x1 = din("x1", [C, HW], f32r)
    x2 = din("x2", [C, HW], f32r)
    y1t = din("y1t", [C, K], f32r)
    y2t = din("y2t", [C, K], f32r)
    wq1t = din("wq1t", [C, D], f32r)
    wq2t = din("wq2t", [C, D], f32r)
    wk1t = din("wk1t", [C, D], f32r)
    wk2t = din("wk2t", [C, D], f32r)
    wv1t = din("wv1t", [C, C], f32r)
    wv2t = din("wv2t", [C, C], f32r)
    bq1 = din("bq1", [D, 1])
    bq2 = din("bq2", [D, 1])
    bk1 = din("bk1", [D, 1])
    bk2 = din("bk2", [D, 1])
    bv1 = din("bv1", [1, C], f32r)
    bv2 = din("bv2", [1, C], f32r)
    ones_r = din("ones_r", [1, K], f32r)
    ones_c = din("ones_c", [K, 1], f32r)
    scol = din("scol", [P, 1])  # scale broadcast to 128 partitions (host)
    o1 = dout("o1", [C, HW])
    o2 = dout("o2", [C, HW])

    # chunked (partition-major) views of the DRAM tensors
    x1r = x1.rearrange("(c p) l -> c p l", p=P)
    x2r = x2.rearrange("(c p) l -> c p l", p=P)
    o1r = o1.rearrange("(c p) l -> c p l", p=P)
    o2r = o2.rearrange("(c p) l -> c p l", p=P)
    y1r = y1t.rearrange("(c p) k -> c p k", p=P)
    y2r = y2t.rearrange("(c p) k -> c p k", p=P)
    wq1r = wq1t.rearrange("(c p) d -> c p d", p=P)
    wq2r = wq2t.rearrange("(c p) d -> c p d", p=P)
    wk1r = wk1t.rearrange("(c p) d -> c p d", p=P)
    wk2r = wk2t.rearrange("(c p) d -> c p d", p=P)
    wv1r = wv1t.rearrange("(c p) e -> c p e", p=P)
    wv2r = wv2t.rearrange("(c p) e -> c p e", p=P)

    with tile.TileContext(nc) as tc, ExitStack() as ctx:
        cpool = ctx.enter_context(tc.tile_pool(name="const", bufs=1))

        # --- load replicated constants -------------------------------------
        def load_chunks(name, src_r, nchunks, width):
            t = cpool.tile([P, nchunks * width], f32r, name=name, tag=name)
            for j in range(nchunks):
                nc.sync.dma_start(t[:, j * width:(j + 1) * width], src_r[j])
            return t

        wq1s = load_chunks("wq1s", wq1r, NCH, D)
        wq2s = load_chunks("wq2s", wq2r, NCH, D)
        wk1s = load_chunks("wk1s", wk1r, NCH, D)
        wk2s = load_chunks("wk2s", wk2r, NCH, D)
        wv1s = load_chunks("wv1s", wv1r, NCH, C)
        wv2s = load_chunks("wv2s", wv2r, NCH, C)
        y1s = load_chunks("y1s", y1r, NCH, K)
        y2s = load_chunks("y2s", y2r, NCH, K)

        def load1(name, src, shape, dt=f32):
            t = cpool.tile(shape, dt, name=name, tag=name)
            nc.sync.dma_start(t[:], src[:])
            return t

        bq1s = load1("bq1s", bq1, [D, 1])
        bq2s = load1("bq2s", bq2, [D, 1])
        bk1s = load1("bk1s", bk1, [D, 1])
        bk2s = load1("bk2s", bk2, [D, 1])
        bv1s = load1("bv1s", bv1, [1, C], f32r)
        bv2s = load1("bv2s", bv2, [1, C], f32r)
        onrs = load1("onrs", ones_r, [1, K], f32r)
        oncs = load1("oncs", ones_c, [K, 1], f32r)
        scols = load1("scols", scol, [P, 1])

        bk2n = cpool.tile([D, 1], f32, name="bk2n", tag="bk2n")
        nc.scalar.mul(bk2n[:], bk2s[:], -1.0)

        # --- setup: K1t (D,K), K2tn = -(K2t+bk2), V1 (K,C), V2 (K,C) -------
        k1s = cpool.tile([D, K], f32r, name="k1s", tag="k1s")
        k2ns = cpool.tile([D, K], f32r, name="k2ns", tag="k2ns")
        v1s = cpool.tile([K, C], f32r, name="v1s", tag="v1s")
        v2s = cpool.tile([K, C], f32r, name="v2s", tag="v2s")

        with ExitStack() as sctx:
            spsum = sctx.enter_context(
                tc.tile_pool(name="spsum", bufs=1, space="PSUM"))

            for (wks, ys, ks, bias, sc) in (
                    (wk1s, y1s, k1s, bk1s, 1.0),
                    (wk2s, y2s, k2ns, bk2n, -1.0)):
                kp = spsum.tile([D, K], f32, name="kp", tag="kp")
                for j in range(NCH):
                    nc.tensor.matmul(
                        kp[:],
                        wks[:, j * D:(j + 1) * D],
                        ys[:, j * K:(j + 1) * K],
                        start=(j == 0), stop=(j == NCH - 1))
                # ks = sc*kp + bias  (sc=-1, bias=-bk2 negates K2t + bk2)
                nc.scalar.activation(ks[:], kp[:], AF.Identity,
                                     bias=bias[:], scale=sc)

            for (ys, wvs, bvs, vs) in (
                    (y1s, wv1s, bv1s, v1s), (y2s, wv2s, bv2s, v2s)):
                vp = spsum.tile([K, C], f32, name="vp", tag="vp")
                for j in range(NCH):
                    nc.tensor.matmul(
                        vp[:],
                        ys[:, j * K:(j + 1) * K],
                        wvs[:, j * C:(j + 1) * C],
                        start=(j == 0), stop=False)
                # += ones.T @ bv  (broadcast bias add over K partitions)
                nc.tensor.matmul(vp[:], onrs[:], bvs[:], start=False,
                                 stop=True)
                nc.scalar.copy(vs[:], vp[:])

        # --- streaming pools ----------------------------------------------
        xpool = ctx.enter_context(tc.tile_pool(name="xpool", bufs=2))
        qsb = ctx.enter_context(tc.tile_pool(name="qsb", bufs=2))
        softp = ctx.enter_context(tc.tile_pool(name="softp", bufs=2))
        opool = ctx.enter_context(tc.tile_pool(name="opool", bufs=4))
        qpp = ctx.enter_context(tc.tile_pool(name="qpp", bufs=1, space="PSUM"))
        epp = ctx.enter_context(tc.tile_pool(name="epp", bufs=1, space="PSUM"))
        upp = ctx.enter_context(tc.tile_pool(name="upp", bufs=3, space="PSUM"))

        for t in range(NT):
            l0 = t * L
            xts = {}
            for s, xr in ((0, x1r), (1, x2r)):
                for j in range(NCH):
                    xt = xpool.tile([P, L], f32r, name=f"x{s}_{j}",
                                    tag=f"x{s}_{j}")
                    nc.sync.dma_start(xt[:], xr[j][:, l0:l0 + L])
                    xts[(s, j)] = xt

            qs = []
            for s, (wqs, bqs) in enumerate(((wq1s, bq1s), (wq2s, bq2s))):
                qp = qpp.tile([D, L], f32, name=f"q{s}p", tag=f"q{s}p")
                for j in range(NCH):
                    nc.tensor.matmul(
                        qp[:],
                        wqs[:, j * D:(j + 1) * D],
                        xts[(s, j)][:],
                        start=(j == 0), stop=(j == NCH - 1))
                q = qsb.tile([D, L], f32r, name=f"q{s}s", tag=f"q{s}s")
                nc.scalar.activation(q[:], qp[:], AF.Identity, bias=bqs[:])
                qs.append(q)

            ep = epp.tile([K, L], f32, name="ep", tag="ep")
            nc.tensor.matmul(ep[:], k1s[:], qs[0][:], start=True, stop=False)
            nc.tensor.matmul(ep[:], k2ns[:], qs[1][:], start=False, stop=True)

            aabs = softp.tile([K, L], f32, name="aabs", tag="aabs")
            nc.scalar.activation(aabs[:], ep[:], AF.Abs)
            expe = softp.tile([K, L], f32r, name="expe", tag="expe")
            nc.scalar.activation(expe[:], aabs[:], AF.Exp)

            sp = epp.tile([1, L], f32, name="sp", tag="sp")
            nc.tensor.matmul(sp[:], oncs[:], expe[:], start=True, stop=True)
            rs = softp.tile([1, L], f32r, name="rs", tag="rs")
            with nc.allow_low_precision(reason="f32r rounding of softmax "
                                        "denominators for the f32r matmul"):
                nc.vector.reciprocal(rs[:], sp[:])
            rbp = epp.tile([K, L], f32, name="rbp", tag="rbp")
            nc.tensor.matmul(rbp[:], onrs[:], rs[:], start=True, stop=True)
            attn = softp.tile([K, L], f32r, name="attn", tag="attn")
            nc.vector.tensor_mul(attn[:], expe[:].bitcast(f32), rbp[:])

            for s, (vs, orr) in enumerate(((v1s, o1r), (v2s, o2r))):
                for j in range(NCH):
                    up = upp.tile([P, L], f32, name="up", tag="up")
                    nc.tensor.matmul(up[:], vs[:, j * P:(j + 1) * P],
                                     attn[:], start=True, stop=True)
                    ot = opool.tile([P, L], f32, name="ot", tag="ot")
                    # ot = (up * scale) + x
                    nc.vector.scalar_tensor_tensor(
                        ot[:], up[:], scols[:], xts[(s, j)][:].bitcast(f32),
                        ALU.mult, ALU.add)
                    nc.sync.dma_start(orr[j][:, l0:l0 + L], ot[:])

    nc.compile()
    return nc


def _get_nc():
    if "nc" not in _CACHE:
        try:
            import concourse  # noqa: F401
        except ImportError:
            import sys
            sys.path.insert(0, "/opt/trn_rl_repo")
        _CACHE["nc"] = _build()
    return _CACHE["nc"]


def _make_in_maps(inputs):
    def f32(a):
        return np.ascontiguousarray(np.asarray(a, dtype=np.float32))

    x1 = f32(inputs["x1"]).reshape(N, C, HW)
    x2 = f32(inputs["x2"]).reshape(N, C, HW)
    y1 = f32(inputs["y1"])
    y2 = f32(inputs["y2"])
    shared = {
        "wq1t": f32(np.asarray(inputs["wq1"]).T),
        "wq2t": f32(np.asarray(inputs["wq2"]).T),
        "wk1t": f32(np.asarray(inputs["wk1"]).T),
        "wk2t": f32(np.asarray(inputs["wk2"]).T),
        "wv1t": f32(np.asarray(inputs["wv1"]).T),
        "wv2t": f32(np.asarray(inputs["wv2"]).T),
        "bq1": f32(inputs["bq1"]).reshape(D, 1),
        "bq2": f32(inputs["bq2"]).reshape(D, 1),
        "bk1": f32(inputs["bk1"]).reshape(D, 1),
        "bk2": f32(inputs["bk2"]).reshape(D, 1),
        "bv1": f32(inputs["bv1"]).reshape(1, C),
        "bv2": f32(inputs["bv2"]).reshape(1, C),
        "ones_r": np.ones((1, K), np.float32),
        "ones_c": np.ones((K, 1), np.float32),
        "scol": np.full((P, 1), np.asarray(inputs["scale"]).reshape(-1)[0],
                        dtype=np.float32),
    }
    in_maps = []
    for i in range(N):
        m = dict(shared)
        m["x1"] = x1[i]
        m["x2"] = x2[i]
        m["y1t"] = f32(y1[i].T)
        m["y2t"] = f32(y2[i].T)
        in_maps.append(m)
    return in_maps


def kernel(**inputs):
    nc = _get_nc()
    from concourse.bass_utils import run_bass_kernel_spmd

    in_maps = _make_in_maps(inputs)
    res = run_bass_kernel_spmd(nc, in_maps, list(range(N))).results
    out1 = np.stack([res[i]["o1"] for i in range(N)]).reshape(N, C, H, W)
    out2 = np.stack([res[i]["o2"] for i in range(N)]).reshape(N, C, H, W)
    return out1, out2


# revision 16
# speedup vs baseline: 1.1265x; 1.1265x over previous
"""Trainium2 Bass kernel for nn_CPAMDec_Mix (dual cross-attention, CPAM decoder).

Math (per batch element n):
    q_i = (wq_i @ x_i + bq_i)            # (D, HW)   1x1 conv query
    k_i = y_i @ wk_i.T + bk_i            # (K, D)    linear key
    v_i = y_i @ wv_i.T + bv_i            # (K, C)    linear value
    e   = | q_1.T k_1.T - q_2.T k_2.T |  # (HW, K)
    a   = softmax_K(e)
    out_i = scale * (v_i.T @ a.T) + x_i  # (C, HW)

Sharding: pure data parallel, one batch element per NeuronCore (N=8, 8 cores).
All weights replicated.  Host-side marshaling pre-transposes the small weight
matrices / y tensors so the contraction dim (C) lands on SBUF partitions.

On-chip layout per core (everything streamed over pixel tiles of L=512):
    E^T (K x L) layout keeps softmax results directly usable as the moving
    operand of the output matmul (contract over K).  Softmax over K (the
    partition dim) is done with ones-matmuls: S = 1.T @ exp(E), then
    R = 1/S broadcast back over K partitions with another ones-matmul.
    exp() needs no max-subtraction: energies are |.| >= 0 and bounded
    (~20 for this operator scale), far from fp32 overflow.
    Matmuls run as float32r (fp32 bits, replicated fast path: 1 PE
    cycle/row for moving >= 256 instead of 4 for plain fp32).  The BIR
    verifier requires every f32r matmul operand to be produced as f32r,
    so matmul-feeding DRAM tensors/tiles are declared f32r end-to-end;
    the residual add reads the x tiles bitcast back to f32 (exact bits).
"""

import numpy as np

N, C, H, W, K = 8, 512, 64, 64, 64
HW = H * W          # 4096
D = C // 4          # 128
L = 512             # pixel tile size
NT = HW // L        # 8 tiles
NCH = C // 128      # 4 contraction chunks
P = 128

_CACHE = {}


def _build():
    from contextlib import ExitStack

    import concourse.tile as tile
    from concourse import bacc, mybir

    f32 = mybir.dt.float32
    f32r = mybir.dt.float32r
    AF = mybir.ActivationFunctionType
    ALU = mybir.AluOpType

    nc = bacc.Bacc("TRN2", target_bir_lowering=False, debug=False)

    def din(name, shape, dt=f32):
        return nc.dram_tensor(name, shape, dt, kind="ExternalInput").ap()

    def dout(name, shape):
        return nc.dram_tensor(name, shape, f32, kind="ExternalOutput").ap()

    x1 = din("x1", [C, HW], f32r)
    x2 = din("x2", [C, HW], f32r)
    y1t = din("y1t", [C, K], f32r)
    y2t = din("y2t", [C, K], f32r)
    wq1t = din("wq1t", [C, D], f32r)
    wq2t = din("wq2t", [C, D], f32r)
    wk1t = din("wk1t", [C, D], f32r)
    wk2t = din("wk2t", [C, D], f32r)
    wv1t = din("wv1t", [C, C], f32r)
    wv2t = din("wv2t", [C, C], f32r)
    bq1 = din("bq1", [D, 1])
    bq2 = din("bq2", [D, 1])
    bk1 = din("bk1", [D, 1])
    bk2 = din("bk2", [D, 1])
    bv1 = din("bv1", [1, C], f32r)
    bv2 = din("bv2", [1, C], f32r)
    ones_r = din("ones_r", [1, K], f32r)
    ones_c = din("ones_c", [K, 1], f32r)
    ones_f = din("ones_f", [1, K])  # fp32 ones for the 1/S broadcast matmul
    scol = din("scol", [P, 1])  # scale broadcast to 128 partitions (host)
    o1 = dout("o1", [C, HW])
    o2 = dout("o2", [C, HW])

    # chunked (partition-major) views of the DRAM tensors
    x1r = x1.rearrange("(c p) l -> c p l", p=P)
    x2r = x2.rearrange("(c p) l -> c p l", p=P)
    o1r = o1.rearrange("(c p) l -> c p l", p=P)
    o2r = o2.rearrange("(c p) l -> c p l", p=P)
    y1r = y1t.rearrange("(c p) k -> c p k", p=P)
    y2r = y2t.rearrange("(c p) k -> c p k", p=P)
    wq1r = wq1t.rearrange("(c p) d -> c p d", p=P)
    wq2r = wq2t.rearrange("(c p) d -> c p d", p=P)
    wk1r = wk1t.rearrange("(c p) d -> c p d", p=P)
    wk2r = wk2t.rearrange("(c p) d -> c p d", p=P)
    wv1r = wv1t.rearrange("(c p) e -> c p e", p=P)
    wv2r = wv2t.rearrange("(c p) e -> c p e", p=P)

    with tile.TileContext(nc) as tc, ExitStack() as ctx:
        cpool = ctx.enter_context(tc.tile_pool(name="const", bufs=1))

        # --- load replicated constants -------------------------------------
        def load_chunks(name, src_r, nchunks, width):
            t = cpool.tile([P, nchunks * width], f32r, name=name, tag=name)
            for j in range(nchunks):
                nc.sync.dma_start(t[:, j * width:(j + 1) * width], src_r[j])
            return t

        wq1s = load_chunks("wq1s", wq1r, NCH, D)
        wq2s = load_chunks("wq2s", wq2r, NCH, D)
        wk1s = load_chunks("wk1s", wk1r, NCH, D)
        wk2s = load_chunks("wk2s", wk2r, NCH, D)
        wv1s = load_chunks("wv1s", wv1r, NCH, C)
        wv2s = load_chunks("wv2s", wv2r, NCH, C)
        y1s = load_chunks("y1s", y1r, NCH, K)
        y2s = load_chunks("y2s", y2r, NCH, K)

        def load1(name, src, shape, dt=f32):
            t = cpool.tile(shape, dt, name=name, tag=name)
            nc.sync.dma_start(t[:], src[:])
            return t

        bq1s = load1("bq1s", bq1, [D, 1])
        bq2s = load1("bq2s", bq2, [D, 1])
        bk1s = load1("bk1s", bk1, [D, 1])
        bk2s = load1("bk2s", bk2, [D, 1])
        bv1s = load1("bv1s", bv1, [1, C], f32r)
        bv2s = load1("bv2s", bv2, [1, C], f32r)
        onrs = load1("onrs", ones_r, [1, K], f32r)
        oncs = load1("oncs", ones_c, [K, 1], f32r)
        onfs = load1("onfs", ones_f, [1, K])
        scols = load1("scols", scol, [P, 1])

        bk2n = cpool.tile([D, 1], f32, name="bk2n", tag="bk2n")
        nc.scalar.mul(bk2n[:], bk2s[:], -1.0)

        # --- setup: K1t (D,K), K2tn = -(K2t+bk2), V1 (K,C), V2 (K,C) -------
        k1s = cpool.tile([D, K], f32r, name="k1s", tag="k1s")
        k2ns = cpool.tile([D, K], f32r, name="k2ns", tag="k2ns")
        v1s = cpool.tile([K, C], f32r, name="v1s", tag="v1s")
        v2s = cpool.tile([K, C], f32r, name="v2s", tag="v2s")

        with ExitStack() as sctx:
            spsum = sctx.enter_context(
                tc.tile_pool(name="spsum", bufs=1, space="PSUM"))

            for (wks, ys, ks, bias, sc) in (
                    (wk1s, y1s, k1s, bk1s, 1.0),
                    (wk2s, y2s, k2ns, bk2n, -1.0)):
                kp = spsum.tile([D, K], f32, name="kp", tag="kp")
                for j in range(NCH):
                    nc.tensor.matmul(
                        kp[:],
                        wks[:, j * D:(j + 1) * D],
                        ys[:, j * K:(j + 1) * K],
                        start=(j == 0), stop=(j == NCH - 1))
                # ks = sc*kp + bias  (sc=-1, bias=-bk2 negates K2t + bk2)
                nc.scalar.activation(ks[:], kp[:], AF.Identity,
                                     bias=bias[:], scale=sc)

            for (ys, wvs, bvs, vs) in (
                    (y1s, wv1s, bv1s, v1s), (y2s, wv2s, bv2s, v2s)):
                vp = spsum.tile([K, C], f32, name="vp", tag="vp")
                for j in range(NCH):
                    nc.tensor.matmul(
                        vp[:],
                        ys[:, j * K:(j + 1) * K],
                        wvs[:, j * C:(j + 1) * C],
                        start=(j == 0), stop=False)
                # += ones.T @ bv  (broadcast bias add over K partitions)
                nc.tensor.matmul(vp[:], onrs[:], bvs[:], start=False,
                                 stop=True)
                nc.scalar.copy(vs[:], vp[:])

        # --- streaming pools ----------------------------------------------
        xpool = ctx.enter_context(tc.tile_pool(name="xpool", bufs=3))
        qsb = ctx.enter_context(tc.tile_pool(name="qsb", bufs=3))
        softp = ctx.enter_context(tc.tile_pool(name="softp", bufs=3))
        opool = ctx.enter_context(tc.tile_pool(name="opool", bufs=3))
        qpp = ctx.enter_context(tc.tile_pool(name="qpp", bufs=1, space="PSUM"))
        epp = ctx.enter_context(tc.tile_pool(name="epp", bufs=2, space="PSUM"))
        spp = ctx.enter_context(tc.tile_pool(name="spp", bufs=1, space="PSUM"))
        upp = ctx.enter_context(tc.tile_pool(name="upp", bufs=2, space="PSUM"))

        for t in range(NT):
            l0 = t * L
            xts = {}
            for s, xr in ((0, x1r), (1, x2r)):
                # per-stream tile holding all 4 channel chunks side by side;
                # loads on the SP ring for stream 0, Activation ring for 1
                ldeng = nc.sync if s == 0 else nc.scalar
                xt = xpool.tile([P, NCH * L], f32r, name=f"x{s}", tag=f"x{s}")
                for j in range(NCH):
                    ldeng.dma_start(xt[:, j * L:(j + 1) * L],
                                    xr[j][:, l0:l0 + L])
                xts[s] = xt

            qs = []
            for s, (wqs, bqs) in enumerate(((wq1s, bq1s), (wq2s, bq2s))):
                qp = qpp.tile([D, L], f32, name=f"q{s}p", tag=f"q{s}p")
                for j in range(NCH):
                    nc.tensor.matmul(
                        qp[:],
                        wqs[:, j * D:(j + 1) * D],
                        xts[s][:, j * L:(j + 1) * L],
                        start=(j == 0), stop=(j == NCH - 1))
                q = qsb.tile([D, L], f32r, name=f"q{s}s", tag=f"q{s}s")
                nc.scalar.activation(q[:], qp[:], AF.Identity, bias=bqs[:])
                qs.append(q)

            ep = epp.tile([K, L], f32, name="ep", tag="ep")
            nc.tensor.matmul(ep[:], k1s[:], qs[0][:], start=True, stop=False)
            nc.tensor.matmul(ep[:], k2ns[:], qs[1][:], start=False, stop=True)

            aabs = softp.tile([K, L], f32, name="aabs", tag="aabs")
            nc.scalar.activation(aabs[:], ep[:], AF.Abs)
            expe = softp.tile([K, L], f32r, name="expe", tag="expe")
            nc.scalar.activation(expe[:], aabs[:], AF.Exp)

            sp = spp.tile([1, L], f32, name="sp", tag="sp")
            nc.tensor.matmul(sp[:], oncs[:], expe[:], start=True, stop=True)
            rs = softp.tile([1, L], f32, name="rs", tag="rs")
            # 1/S at ~18 bits; S in [K, K*exp(~20)] so no edge cases
            nc.vector.reciprocal_approx_fast(rs[:], sp[:])
            rbp = spp.tile([K, L], f32, name="rbp", tag="rbp")
            # fp32 (non-f32r) broadcast matmul: rs is DVE-produced fp32
            nc.tensor.matmul(rbp[:], onfs[:], rs[:], start=True, stop=True)
            attn = softp.tile([K, L], f32r, name="attn", tag="attn")
            nc.vector.tensor_mul(attn[:], expe[:].bitcast(f32), rbp[:])

            for s, (vs, orr) in enumerate(((v1s, o1r), (v2s, o2r))):
                eng = nc.vector if s == 0 else nc.gpsimd
                steng = nc.sync if s == 0 else nc.scalar
                ot = opool.tile([P, NCH * L], f32, name=f"ot{s}", tag=f"ot{s}")
                for j in range(NCH):
                    up = upp.tile([P, L], f32, name="up", tag="up")
                    nc.tensor.matmul(up[:], vs[:, j * P:(j + 1) * P],
                                     attn[:], start=True, stop=True)
                    if s == 0:
                        # ot = (up * scale) + x in one DVE op
                        eng.scalar_tensor_tensor(
                            ot[:, j * L:(j + 1) * L], up[:], scols[:],
                            xts[s][:, j * L:(j + 1) * L].bitcast(f32),
                            ALU.mult, ALU.add)
                    else:
                        # GPSIMD cannot read PSUM (and Pool lacks
                        # TensorScalarPtr): scale on ACT, add on GPSIMD
                        us = qsb.tile([P, L], f32, name="us", tag="us")
                        nc.scalar.activation(us[:], up[:], AF.Copy,
                                             scale=scols[:])
                        eng.tensor_add(
                            ot[:, j * L:(j + 1) * L], us[:],
                            xts[s][:, j * L:(j + 1) * L].bitcast(f32))
                    steng.dma_start(orr[j][:, l0:l0 + L],
                                    ot[:, j * L:(j + 1) * L])

    nc.compile()
    return nc


def _get_nc():
    if "nc" not in _CACHE:
        try:
            import concourse  # noqa: F401
        except ImportError:
            import sys
            sys.path.insert(0, "/opt/trn_rl_repo")
        _CACHE["nc"] = _build()
    return _CACHE["nc"]


def _make_in_maps(inputs):
    def f32(a):
        return np.ascontiguousarray(np.asarray(a, dtype=np.float32))

    x1 = f32(inputs["x1"]).reshape(N, C, HW)
    x2 = f32(inputs["x2"]).reshape(N, C, HW)
    y1 = f32(inputs["y1"])
    y2 = f32(inputs["y2"])
    shared = {
        "wq1t": f32(np.asarray(inputs["wq1"]).T),
        "wq2t": f32(np.asarray(inputs["wq2"]).T),
        "wk1t": f32(np.asarray(inputs["wk1"]).T),
        "wk2t": f32(np.asarray(inputs["wk2"]).T),
        "wv1t": f32(np.asarray(inputs["wv1"]).T),
        "wv2t": f32(np.asarray(inputs["wv2"]).T),
        "bq1": f32(inputs["bq1"]).reshape(D, 1),
        "bq2": f32(inputs["bq2"]).reshape(D, 1),
        "bk1": f32(inputs["bk1"]).reshape(D, 1),
        "bk2": f32(inputs["bk2"]).reshape(D, 1),
        "bv1": f32(inputs["bv1"]).reshape(1, C),
        "bv2": f32(inputs["bv2"]).reshape(1, C),
        "ones_r": np.ones((1, K), np.float32),
        "ones_c": np.ones((K, 1), np.float32),
        "ones_f": np.ones((1, K), np.float32),
        "scol": np.full((P, 1), np.asarray(inputs["scale"]).reshape(-1)[0],
                        dtype=np.float32),
    }
    in_maps = []
    for i in range(N):
        m = dict(shared)
        m["x1"] = x1[i]
        m["x2"] = x2[i]
        m["y1t"] = f32(y1[i].T)
        m["y2t"] = f32(y2[i].T)
        in_maps.append(m)
    return in_maps


def kernel(**inputs):
    nc = _get_nc()
    from concourse.bass_utils import run_bass_kernel_spmd

    in_maps = _make_in_maps(inputs)
    res = run_bass_kernel_spmd(nc, in_maps, list(range(N))).results
    out1 = np.stack([res[i]["o1"] for i in range(N)]).reshape(N, C, H, W)
    out2 = np.stack([res[i]["o2"] for i in range(N)]).reshape(N, C, H, W)
    return out1, out2
